# revision 1
# baseline (speedup 1.0000x reference)
"""Trainium2 Bass kernel for nn_Block_77318001263203 (dense transformer block).

Distribution over 8 NeuronCores: data-parallel over batch (2 groups of 4
cores) x tensor-parallel over heads (4 heads/core) for attention+proj,
ReduceScatter of the proj partials over each 4-core group (token axis ==
partition axis, so rank r directly receives its 512-token slice), then
token-parallel FFN with full (replicated) FFN weights — no second
collective. All matmuls run as float32r (full PE rate, ~1.2e-4 rel err).

kernel(**inputs) takes the FULL inputs from setup_inputs() and returns the
FULL [2, 2048, 1024] output.
"""

import numpy as np

import concourse.bass as bass
import concourse.mybir as mybir
import concourse.tile as tile
from concourse import bacc
from concourse.bass_utils import run_bass_kernel_spmd
from concourse.masks import make_identity

# problem dims (hardcoded per the harness contract)
B, S, D = 2, 2048, 1024
H, HS, F = 16, 64, 4096
EPS = 1e-5
P = 128
NCORES = 8
TP = 4  # cores per batch group
HPC = H // TP  # heads per core = 4
SL = S // TP  # token slice per core = 512
QT = 512  # query tile
KB = 128  # key block
NEG = -1.0e9  # additive causal mask (exp underflows to exactly 0)

f32 = mybir.dt.float32
f32r = mybir.dt.float32r

REPLICA_GROUPS = [[0, 1, 2, 3], [4, 5, 6, 7]]


def _bcast_row_ap(t, row, width):
    """DMA-source AP broadcasting row `row` of DRAM tensor t to 128 partitions."""
    return bass.AP(tensor=t, offset=row * width, ap=[[0, P], [1, width]])


def build_bass():
    nc = bacc.Bacc("TRN2", target_bir_lowering=False, debug=False, num_devices=NCORES)

    xT = nc.dram_tensor("xT", [D, S], f32, kind="ExternalInput").ap()
    xs = nc.dram_tensor("xs", [SL, D], f32, kind="ExternalInput").ap()
    wq2 = nc.dram_tensor("wq2", [D, HPC * HS], f32, kind="ExternalInput").ap()
    wk2 = nc.dram_tensor("wk2", [D, HPC * HS], f32, kind="ExternalInput").ap()
    wv4 = nc.dram_tensor("wv4", [D, HPC * HS], f32, kind="ExternalInput").ap()
    wp = nc.dram_tensor("wp", [HPC * HS, D], f32, kind="ExternalInput").ap()
    w1 = nc.dram_tensor("w1", [D, F], f32, kind="ExternalInput").ap()
    w2 = nc.dram_tensor("w2", [F, D], f32, kind="ExternalInput").ap()
    cvec = nc.dram_tensor("cvec", [6, D], f32, kind="ExternalInput").ap()
    b1d = nc.dram_tensor("b1d", [F], f32, kind="ExternalInput").ap()
    out = nc.dram_tensor("out", [SL, D], f32, kind="ExternalOutput").ap()

    # collective bounce buffers (internal DRAM)
    rs_in = nc.dram_tensor("rs_in", [S, D], f32)
    rs_out = nc.dram_tensor("rs_out", [SL, D], f32)

    # additive causal masks for the 4 diagonal offsets of a 512-query tile
    m_np = np.zeros((4, KB, QT), dtype=np.float32)
    for m in range(4):
        t_idx = np.arange(KB)[:, None] + m * KB
        q_idx = np.arange(QT)[None, :]
        m_np[m] = np.where(t_idx <= q_idx, 0.0, NEG)
    masks_dram = nc.inline_tensor(m_np, name="causal_masks")

    with tile.TileContext(nc) as tc:
        with tc.tile_pool(name="const", bufs=1) as constp:
            ident_f = constp.tile([P, P], f32)
            make_identity(nc, ident_f)
            ident = constp.tile([P, P], f32r)
            nc.vector.tensor_copy(ident, ident_f)
            eps_t = constp.tile([P, 1], f32)
            nc.vector.memset(eps_t, EPS)
            b1_sb = constp.tile([P, F // P], f32)
            nc.sync.dma_start(b1_sb, b1d.rearrange("(ko p) -> p ko", p=P))
            g2b = constp.tile([P, D], f32)
            nc.gpsimd.dma_start(g2b, _bcast_row_ap(cvec.tensor, 3, D))
            be2b = constp.tile([P, D], f32)
            nc.gpsimd.dma_start(be2b, _bcast_row_ap(cvec.tensor, 4, D))
            b2b = constp.tile([P, D], f32)
            nc.gpsimd.dma_start(b2b, _bcast_row_ap(cvec.tensor, 5, D))

            # ---------------- Phase A: QKV + attention + proj ----------------
            with (
                tc.tile_pool(name="wqkvp", bufs=1) as wqkvp,
                tc.tile_pool(name="qkvo", bufs=1) as qkvo,
                tc.tile_pool(name="xrp", bufs=2) as xrp,
                tc.tile_pool(name="smallp", bufs=4) as smallp,
                tc.tile_pool(name="projp", bufs=3) as projp,
            ):
                wq_sb = wqkvp.tile([P, D // P, HPC * HS], f32r, tag="wq")
                nc.sync.dma_start(
                    wq_sb, wq2.rearrange("(ko p) m -> p ko m", p=P).bitcast(f32r)
                )
                wk_sb = wqkvp.tile([P, D // P, HPC * HS], f32r, tag="wk")
                nc.sync.dma_start(
                    wk_sb, wk2.rearrange("(ko p) m -> p ko m", p=P).bitcast(f32r)
                )
                wv_sb = wqkvp.tile([P, D // P, HPC * HS], f32r, tag="wv")
                nc.sync.dma_start(
                    wv_sb, wv4.rearrange("(ko p) m -> p ko m", p=P).bitcast(f32r)
                )
                wp_sb = wqkvp.tile([P, (HPC * HS) // P, D], f32r, tag="wp")
                nc.sync.dma_start(
                    wp_sb, wp.rearrange("(ko p) n -> p ko n", p=P).bitcast(f32r)
                )
                masks_sb = wqkvp.tile([P, 4, QT], f32, tag="masks")
                for m in range(4):
                    nc.sync.dma_start(masks_sb[:, m, :], masks_dram.ap()[m])

                # outputs of QKV: qT/kT per head pair, v (+ones col) per head
                q2T = qkvo.tile([P, 2, S], f32r, tag="q2T")
                k2T = qkvo.tile([P, 2, S], f32r, tag="k2T")
                v4e = qkvo.tile([P, S // P, HPC * (HS + 1)], f32r, tag="v4e")
                attnT = qkvo.tile([P, 2, S], f32r, tag="attnT")
                ones4 = qkvo.tile([P, HPC, 1], f32, tag="ones4")
                nc.vector.memset(ones4, 1.0)

                with tc.tile_pool(name="ps_qkv", bufs=3, space="PSUM") as psq:
                    for tt in range(S // QT):
                        xr = xrp.tile([P, D // P, QT], f32r, tag="xr")
                        nc.sync.dma_start(
                            xr,
                            xT[:, tt * QT : (tt + 1) * QT]
                            .rearrange("(ko p) m -> p ko m", p=P)
                            .bitcast(f32r),
                        )
                        for hp in range(2):
                            qps = psq.tile([P, QT], f32, tag="qk")
                            for ko in range(D // P):
                                nc.tensor.matmul(
                                    qps,
                                    wq_sb[:, ko, hp * P : (hp + 1) * P],
                                    xr[:, ko, :],
                                    start=(ko == 0),
                                    stop=(ko == D // P - 1),
                                )
                            nc.vector.tensor_copy(
                                q2T[:, hp, tt * QT : (tt + 1) * QT], qps
                            )
                            kps = psq.tile([P, QT], f32, tag="qk")
                            for ko in range(D // P):
                                nc.tensor.matmul(
                                    kps,
                                    wk_sb[:, ko, hp * P : (hp + 1) * P],
                                    xr[:, ko, :],
                                    start=(ko == 0),
                                    stop=(ko == D // P - 1),
                                )
                            nc.vector.tensor_copy(
                                k2T[:, hp, tt * QT : (tt + 1) * QT], kps
                            )
                        for mt in range(QT // P):
                            vps = psq.tile([P, HPC * HS], f32, tag="v")
                            for ko in range(D // P):
                                nc.tensor.matmul(
                                    vps,
                                    xr[:, ko, mt * P : (mt + 1) * P],
                                    wv_sb[:, ko, :],
                                    start=(ko == 0),
                                    stop=(ko == D // P - 1),
                                )
                            idx = tt * (QT // P) + mt
                            vv = v4e[:, idx, :].rearrange("p (h e) -> p h e", e=HS + 1)
                            nc.vector.tensor_copy(
                                vv[:, :, 0:HS],
                                vps.rearrange("p (h e) -> p h e", e=HS),
                            )
                            nc.vector.tensor_copy(vv[:, :, HS : HS + 1], ones4)

                with (
                    tc.tile_pool(name="ps_sc", bufs=4, space="PSUM") as pssc,
                    tc.tile_pool(name="ps_at", bufs=2, space="PSUM") as psat,
                    tc.tile_pool(name="ps_pr", bufs=2, space="PSUM") as pspr,
                ):
                    for qt in range(S // QT):
                        nkb = 4 * qt + 4
                        for hp in range(2):
                            ape = psat.tile([HS + 1, QT], f32, tag="at")
                            apo = psat.tile([HS + 1, QT], f32, tag="at")
                            for kb in range(nkb):
                                qsl = slice(qt * QT, (qt + 1) * QT)
                                ksl = slice(kb * KB, (kb + 1) * KB)
                                spe = pssc.tile([P, QT], f32, tag="sc")
                                spo = pssc.tile([P, QT], f32, tag="sc")
                                nc.tensor.matmul(
                                    spe,
                                    k2T[0:HS, hp, ksl],
                                    q2T[0:HS, hp, qsl],
                                    start=True,
                                    stop=True,
                                    tile_position=(0, 0),
                                )
                                nc.tensor.matmul(
                                    spo,
                                    k2T[HS : 2 * HS, hp, ksl],
                                    q2T[HS : 2 * HS, hp, qsl],
                                    start=True,
                                    stop=True,
                                    tile_position=(64, 0),
                                )
                                moff = kb - 4 * qt
                                if moff >= 0:
                                    nc.vector.tensor_add(spe, spe, masks_sb[:, moff, :])
                                    nc.vector.tensor_add(spo, spo, masks_sb[:, moff, :])
                                ee = smallp.tile([P, QT], f32r, tag="ee")
                                eo = smallp.tile([P, QT], f32r, tag="eo")
                                nc.scalar.activation(
                                    out=ee,
                                    in_=spe,
                                    func=mybir.ActivationFunctionType.Exp,
                                    scale=float(HS) ** -0.5,
                                )
                                nc.scalar.activation(
                                    out=eo,
                                    in_=spo,
                                    func=mybir.ActivationFunctionType.Exp,
                                    scale=float(HS) ** -0.5,
                                )
                                he = (2 * hp) * (HS + 1)
                                ho = (2 * hp + 1) * (HS + 1)
                                nc.tensor.matmul(
                                    ape,
                                    v4e[:, kb, he : he + HS + 1],
                                    ee,
                                    start=(kb == 0),
                                    stop=(kb == nkb - 1),
                                )
                                nc.tensor.matmul(
                                    apo,
                                    v4e[:, kb, ho : ho + HS + 1],
                                    eo,
                                    start=(kb == 0),
                                    stop=(kb == nkb - 1),
                                )
                            # normalize: rows 0..63 / row 64
                            qsl = slice(qt * QT, (qt + 1) * QT)
                            for par, aps in ((0, ape), (1, apo)):
                                rec = smallp.tile([1, QT], f32, tag="rec")
                                nc.vector.reciprocal(rec, aps[HS : HS + 1, :])
                                bc = smallp.tile([HS, QT], f32, tag="bc")
                                nc.gpsimd.partition_broadcast(bc, rec)
                                nc.vector.tensor_mul(
                                    attnT[par * HS : (par + 1) * HS, hp, qsl],
                                    aps[0:HS, :],
                                    bc,
                                )
                        # proj for this qt's 4 token tiles of 128
                        for mt in range(4 * qt, 4 * qt + 4):
                            prj = projp.tile([P, D], f32, tag="prj")
                            for nh in range(D // QT):
                                pps = pspr.tile([P, QT], f32, tag="pr")
                                for ko in range(2):
                                    nc.tensor.matmul(
                                        pps,
                                        attnT[:, ko, mt * P : (mt + 1) * P],
                                        wp_sb[:, ko, nh * QT : (nh + 1) * QT],
                                        start=(ko == 0),
                                        stop=(ko == 1),
                                    )
                                nc.vector.tensor_copy(
                                    prj[:, nh * QT : (nh + 1) * QT], pps
                                )
                            nc.sync.dma_start(rs_in.ap()[mt * P : (mt + 1) * P, :], prj)

            # ---------------- ReduceScatter over the 4-core group ----------------
            nc.gpsimd.collective_compute(
                "ReduceScatter",
                mybir.AluOpType.add,
                replica_groups=REPLICA_GROUPS,
                ins=[rs_in.ap().opt()],
                outs=[rs_out.ap().opt()],
            )

            # ---------------- Phase B: LN1 + FFN + LN2 ----------------
            with tc.tile_pool(name="ffn_keep", bufs=1) as keep:
                x1r = keep.tile([P, SL // P, D], f32r, tag="x1r")
                x1T = keep.tile([P, D // P, SL], f32r, tag="x1T")
                hT = keep.tile([P, F // P, SL], f32r, tag="hT")

                with (
                    tc.tile_pool(name="ln1p", bufs=2) as ln1p,
                    tc.tile_pool(name="ln1c", bufs=1) as ln1c,
                    tc.tile_pool(name="ps_tr", bufs=2, space="PSUM") as pstr,
                ):
                    g1b = ln1c.tile([P, D], f32, tag="g1b")
                    nc.gpsimd.dma_start(g1b, _bcast_row_ap(cvec.tensor, 1, D))
                    be1b = ln1c.tile([P, D], f32, tag="be1b")
                    nc.gpsimd.dma_start(be1b, _bcast_row_ap(cvec.tensor, 2, D))
                    bpb = ln1c.tile([P, D], f32, tag="bpb")
                    nc.gpsimd.dma_start(bpb, _bcast_row_ap(cvec.tensor, 0, D))

                    for st in range(SL // P):
                        y = ln1p.tile([P, D], f32, tag="y")
                        nc.sync.dma_start(y, rs_out.ap()[st * P : (st + 1) * P, :])
                        xst = ln1p.tile([P, D], f32, tag="xst")
                        nc.sync.dma_start(xst, xs[st * P : (st + 1) * P, :])
                        nc.vector.tensor_add(y, y, xst)
                        nc.vector.tensor_add(y, y, bpb)
                        stats = ln1p.tile([P, 2, 6], f32, tag="stats")
                        yv = y.rearrange("p (s d) -> p s d", s=2)
                        nc.vector.bn_stats(out=stats[:, 0, :], in_=yv[:, 0, :])
                        nc.vector.bn_stats(out=stats[:, 1, :], in_=yv[:, 1, :])
                        mv = ln1p.tile([P, 2], f32, tag="mv")
                        nc.vector.bn_aggr(out=mv, in_=stats)
                        rstd = ln1p.tile([P, 1], f32, tag="rstd")
                        nc.scalar.activation(
                            out=rstd,
                            in_=mv[:, 1:2],
                            func=mybir.ActivationFunctionType.Sqrt,
                            bias=eps_t,
                            scale=1.0,
                        )
                        nc.vector.reciprocal(rstd, rstd)
                        tmp = ln1p.tile([P, D], f32, tag="tmp")
                        nc.vector.tensor_scalar(
                            out=tmp,
                            in0=y,
                            scalar1=mv[:, 0:1],
                            scalar2=rstd,
                            op0=mybir.AluOpType.subtract,
                            op1=mybir.AluOpType.mult,
                        )
                        nc.vector.tensor_mul(tmp, tmp, g1b)
                        nc.vector.tensor_add(x1r[:, st, :], tmp, be1b)
                        # transpose this token tile into x1T
                        for dk in range(D // P):
                            tp = pstr.tile([P, P], f32r, tag="tp")
                            nc.tensor.transpose(
                                tp, x1r[:, st, dk * P : (dk + 1) * P], ident
                            )
                            nc.vector.tensor_copy(
                                x1T[:, dk, st * P : (st + 1) * P], tp
                            )

                # FFN first matmul: hT[f, tok] = w1.T @ x1T, relu(+b1) fused
                with (
                    tc.tile_pool(name="w1p", bufs=3) as w1p,
                    tc.tile_pool(name="ps_h", bufs=2, space="PSUM") as psh,
                ):
                    for ft in range(F // P):
                        w1t = w1p.tile([P, D // P, P], f32r, tag="w1t")
                        nc.sync.dma_start(
                            w1t,
                            w1[:, ft * P : (ft + 1) * P]
                            .rearrange("(ko p) m -> p ko m", p=P)
                            .bitcast(f32r),
                        )
                        hps = psh.tile([P, SL], f32, tag="h")
                        for ko in range(D // P):
                            nc.tensor.matmul(
                                hps,
                                w1t[:, ko, :],
                                x1T[:, ko, :],
                                start=(ko == 0),
                                stop=(ko == D // P - 1),
                            )
                        nc.scalar.activation(
                            out=hT[:, ft, :],
                            in_=hps,
                            func=mybir.ActivationFunctionType.Relu,
                            bias=b1_sb[:, ft : ft + 1],
                            scale=1.0,
                        )

                # FFN second matmul (directly in [tok, d] layout) + residual + LN2
                with (
                    tc.tile_pool(name="w2p", bufs=2) as w2p,
                    tc.tile_pool(name="zp", bufs=1) as zp,
                    tc.tile_pool(name="ln2p", bufs=2) as ln2p,
                    tc.tile_pool(name="ps_y", bufs=2, space="PSUM") as psy,
                ):
                    NQ = 256  # d-quarter width
                    z = zp.tile([P, SL // P, D], f32, tag="z")
                    for dtq in range(D // NQ):
                        for kh in range(2):
                            w2t = w2p.tile([P, 16, NQ], f32r, tag="w2t")
                            nc.sync.dma_start(
                                w2t,
                                w2[kh * 2048 : (kh + 1) * 2048, dtq * NQ : (dtq + 1) * NQ]
                                .rearrange("(ko p) n -> p ko n", p=P)
                                .bitcast(f32r),
                            )
                            for mt in range(SL // P):
                                yps = psy.tile([P, NQ], f32, tag="yq")
                                for ko in range(16):
                                    nc.tensor.matmul(
                                        yps,
                                        hT[:, kh * 16 + ko, mt * P : (mt + 1) * P],
                                        w2t[:, ko, :],
                                        start=(ko == 0),
                                        stop=(ko == 15),
                                    )
                                dsl = slice(dtq * NQ, (dtq + 1) * NQ)
                                if kh == 0:
                                    # stash first-half partial into z
                                    nc.vector.tensor_copy(z[:, mt, dsl], yps)
                                else:
                                    nc.vector.tensor_add(z[:, mt, dsl], z[:, mt, dsl], yps)
                    # z += x1 + b2, then LN2 -> out
                    for mt in range(SL // P):
                        zm = z[:, mt, :]
                        nc.vector.tensor_add(zm, zm, x1r[:, mt, :])
                        nc.vector.tensor_add(zm, zm, b2b)
                        stats = ln2p.tile([P, 2, 6], f32, tag="stats2")
                        zv = zm.rearrange("p (s d) -> p s d", s=2)
                        nc.vector.bn_stats(out=stats[:, 0, :], in_=zv[:, 0, :])
                        nc.vector.bn_stats(out=stats[:, 1, :], in_=zv[:, 1, :])
                        mv = ln2p.tile([P, 2], f32, tag="mv2")
                        nc.vector.bn_aggr(out=mv, in_=stats)
                        rstd = ln2p.tile([P, 1], f32, tag="rstd2")
                        nc.scalar.activation(
                            out=rstd,
                            in_=mv[:, 1:2],
                            func=mybir.ActivationFunctionType.Sqrt,
                            bias=eps_t,
                            scale=1.0,
                        )
                        nc.vector.reciprocal(rstd, rstd)
                        o = ln2p.tile([P, D], f32, tag="o")
                        nc.vector.tensor_scalar(
                            out=o,
                            in0=zm,
                            scalar1=mv[:, 0:1],
                            scalar2=rstd,
                            op0=mybir.AluOpType.subtract,
                            op1=mybir.AluOpType.mult,
                        )
                        nc.vector.tensor_mul(o, o, g2b)
                        nc.vector.tensor_add(o, o, be2b)
                        nc.sync.dma_start(out[mt * P : (mt + 1) * P, :], o)

    nc.compile()
    return nc


_NC_CACHE = []


def _get_nc():
    if not _NC_CACHE:
        _NC_CACHE.append(build_bass())
    return _NC_CACHE[0]


def make_in_maps(x, wq, wk, wv, w_proj, b_proj, w1, b1, w2, b2, g1, be1, g2, be2):
    x = np.asarray(x, dtype=np.float32)
    cat = lambda w, h0: np.ascontiguousarray(
        np.concatenate([np.asarray(w[h0 + i], dtype=np.float32) for i in range(HPC)], axis=1)
    )
    cvec_rows = [b_proj, g1, be1, g2, be2, b2]
    cvec = np.ascontiguousarray(np.stack([np.asarray(v, dtype=np.float32) for v in cvec_rows]))
    w1c = np.ascontiguousarray(np.asarray(w1, dtype=np.float32))
    w2c = np.ascontiguousarray(np.asarray(w2, dtype=np.float32))
    b1c = np.ascontiguousarray(np.asarray(b1, dtype=np.float32))
    wpc = np.ascontiguousarray(np.asarray(w_proj, dtype=np.float32))
    xTs = [np.ascontiguousarray(x[g].T) for g in range(B)]
    in_maps = []
    for c in range(NCORES):
        g, r = divmod(c, TP)
        h0 = HPC * r
        in_maps.append(
            {
                "xT": xTs[g],
                "xs": np.ascontiguousarray(x[g, SL * r : SL * (r + 1)]),
                "wq2": cat(wq, h0),
                "wk2": cat(wk, h0),
                "wv4": cat(wv, h0),
                "wp": np.ascontiguousarray(wpc[HPC * HS * r : HPC * HS * (r + 1)]),
                "w1": w1c,
                "w2": w2c,
                "cvec": cvec,
                "b1d": b1c,
            }
        )
    return in_maps


def assemble(results):
    full = np.empty((B, S, D), dtype=np.float32)
    for c in range(NCORES):
        g, r = divmod(c, TP)
        full[g, SL * r : SL * (r + 1)] = results[c]["out"]
    return full


def kernel(**inputs):
    nc = _get_nc()
    in_maps = make_in_maps(**inputs)
    res = run_bass_kernel_spmd(nc, in_maps, core_ids=list(range(NCORES)))
    return assemble(res.results)
